# revision 17
# baseline (speedup 1.0000x reference)
"""TRN2 Bass kernel for nn_BiDecoder (GNN edge rating decoder), 8 NeuronCores.

ratings[e] = sum_r softmax_r(ufeat[src[e]] @ Ps[r] @ ifeat[dst[e]]) * (r+1)

v2 design:
  - Edges dst-sorted into 8 contiguous shards (each core owns an item band).
  - Within a core, items are covered by NW aligned 128-item windows; edges are
    grouped by (window w, src-quarter q) and sorted by dst inside each group.
    Each group gets G slots (G multiple of 256); trailing pads idx=-1.
  - Item side needs NO gather: Y[j, (r,d)] = sum_f Ps[r,d,f] ifeat[j,f] is
    precomputed per window on PE (fp16); per tile a one-hot SelT (built with
    tensor_mask_reduce run-intervals, since dst-sorted edges give each item a
    contiguous slot run) expands Y rows to edges via one 320-col matmul.
  - User side: dma_gather of ufeat rows, 4 src-quarters (int16 idx < 25000)
    issued round-robin on 4 SWDGE queues so descriptor generation runs on all
    Q7 cpu pairs in parallel.
  - scores = sum_d us*yv via fp16 DVE mul (2x) + binary-tree adds (2x),
    softmax (no max-sub; scores bounded) -> ratings.
"""
import sys

sys.path.insert(0, "/opt/trn_rl_repo")
import numpy as np

P = 128
D = 64
R = 5
RD = R * D
N_USERS, N_ITEMS, E = 100000, 50000, 1000000
N_CORES = 8
E_CORE = E // N_CORES
NQ = 5  # overlapping int16-addressable src groups
UQ_BASE = (0, 16808, 33616, 50424, 67232)
UQ_LEN = (32768, 32768, 32768, 32768, 32768)
C_TARGET = 2500

_NC_CACHE = {}


def _plan(src, dst):
    """Shard edges; greedy equal-count windows (item span <= 128, cut mid-item
    is fine since SelT is host-built); balanced src-quarters with overlapping
    int16-addressable ranges. Returns per-core layout + global (NW, G)."""
    perm = np.argsort(dst, kind="stable")
    shards = perm.reshape(N_CORES, E_CORE)
    A = np.array(UQ_BASE)
    cores = []
    maxw = 0
    maxg = 0
    for c in range(N_CORES):
        eid = shards[c]
        d = dst[eid].astype(np.int64)
        s = src[eid].astype(np.int64)
        it0s = []
        bounds = [0]
        i = 0
        while i < E_CORE:
            it0 = int(d[i])
            j = min(
                i + C_TARGET,
                E_CORE,
                int(np.searchsorted(d, it0 + P, side="left")),
            )
            it0s.append(it0)
            bounds.append(j)
            i = j
        nw = len(it0s)
        w = np.zeros(E_CORE, np.int64)
        for wi in range(nw):
            w[bounds[wi] : bounds[wi + 1]] = wi
        it0_arr = np.array(it0s, np.int64)
        q = np.zeros(E_CORE, np.int64)
        for wi in range(nw):
            lo, hi = bounds[wi], bounds[wi + 1]
            ss = s[lo:hi]
            n = hi - lo
            q_hi = np.searchsorted(A, ss, side="right") - 1
            can_lo = (q_hi > 0) & (ss <= A[np.maximum(q_hi - 1, 0)] + 32767)
            forced = np.bincount(q_hi[~can_lo], minlength=NQ)
            flexz = np.bincount(q_hi[can_lo] - 1, minlength=NQ)[: NQ - 1]
            T = -(-n // NQ)
            qq = q_hi.copy()
            inbound = 0
            for z in range(NQ - 1):
                x = int(np.clip(T - forced[z] - inbound, 0, flexz[z]))
                idxz = np.where(can_lo & (q_hi - 1 == z))[0]
                qq[idxz[:x]] = z
                inbound = flexz[z] - x
            q[lo:hi] = qq
        g = w * NQ + q
        order = np.argsort(g, kind="stable")
        eid, d, s, w, q, g = (a[order] for a in (eid, d, s, w, q, g))
        cnt = np.bincount(g, minlength=nw * NQ)
        maxg = max(maxg, int(cnt.max()))
        maxw = max(maxw, nw)
        cores.append((eid, d, s, w, q, g, it0_arr, nw, cnt))
    NW = maxw
    G = max(128, -(-maxg // 128) * 128)
    return cores, NW, G


def _prepare(ufeat, ifeat, Ps, src, dst):
    ufeat = np.asarray(ufeat, np.float32)
    ifeat = np.asarray(ifeat, np.float32)
    Ps = np.asarray(Ps, np.float32)
    src = np.asarray(src)
    dst = np.asarray(dst)
    cores, NW, G = _plan(src, dst)
    _NC_CACHE["params"] = (NW, G)
    S16 = G // 16
    NG = NW * NQ
    p2 = np.ascontiguousarray(Ps.transpose(2, 0, 1).reshape(D, RD)).astype(np.float16)
    vals = np.tile(np.arange(1.0, 6.0, dtype=np.float32), (P, 1))
    uf = np.zeros((N_USERS, 2 * D), np.float16)
    uf[:, :D] = ufeat.astype(np.float16)
    in_maps, metas = [], []
    A = np.array(UQ_BASE)
    for (eid, d, s, w, q, g, it0_arr, nw, cnt) in cores:
        nslot = NG * G
        starts = np.zeros(NG + 1, np.int64)
        starts[1 : len(cnt) + 1] = np.cumsum(cnt)
        pos = np.arange(E_CORE) - starts[g]
        slot = g * G + pos
        idxu = np.zeros(nslot, np.int16)
        idxu[slot] = (s - A[q]).astype(np.int16)
        slot2eid = np.full(nslot, -1, np.int64)
        slot2eid[slot] = eid
        dl = d - it0_arr[w]  # 0..127 within window slab
        assert dl.min() >= 0 and dl.max() < P
        # host-built one-hot SelT: [row j, slot] = 1 iff slot's dst-local == j
        selt_h = np.zeros((P, nslot), np.float16)
        selt_h[dl, slot] = 1.0
        wrapped = (
            idxu.reshape(NG, S16, 16).transpose(2, 0, 1).reshape(16, NG * S16)
        )  # [16, (group-major, col-minor)]; element i of group = [i%16, i//16]
        wrapped = np.tile(wrapped, (8, 1)).astype(np.int16)
        slabs = np.zeros((NW * P, D), np.float32)
        for wi in range(nw):
            it0 = int(it0_arr[wi])
            navail = min(P, N_ITEMS - it0)
            slabs[wi * P : wi * P + navail] = ifeat[it0 : it0 + navail]
        ifT = np.ascontiguousarray(slabs.T).astype(np.float16)
        in_maps.append(
            {
                "ufeat": uf,
                "ifT": ifT,
                "p2": p2,
                "idxu": wrapped,
                "selt": np.ascontiguousarray(
                    selt_h.reshape(P, NW, NQ * G).transpose(1, 0, 2).reshape(
                        NW * P, NQ * G
                    )
                ),
                "vals": vals,
            }
        )
        metas.append(slot2eid)
    return in_maps, metas


def _emit(nc, tc, aps, NW, G):
    import concourse.mybir as mybir
    from concourse import library_config

    f32, f16, i16 = mybir.dt.float32, mybir.dt.float16, mybir.dt.int16
    AF = mybir.ActivationFunctionType
    ALU = mybir.AluOpType
    AX = mybir.AxisListType
    TPW = NQ * G // P
    NG = NW * NQ
    S16 = G // 16
    GT = G // P  # tiles per group
    ufeat_d, ifT_d, p2_d, idxu_d, selt_d, vals_d, out_d = aps

    nc.gpsimd.load_library(library_config.mlp)
    with tc.tile_pool(name="const", bufs=1) as cpool:
        p2_sb = cpool.tile([D, RD], f16)
        nc.sync.dma_start(p2_sb[:], p2_d[:])
        ifT_sb = cpool.tile([D, NW * P], f16)
        nc.sync.dma_start(ifT_sb[:], ifT_d[:])
        idx_sb = cpool.tile([P, NG * S16], i16)
        nc.sync.dma_start(idx_sb[:], idxu_d[:])
        vals_sb = cpool.tile([P, R], f32)
        nc.sync.dma_start(vals_sb[:], vals_d[:])
        ysb = cpool.tile([P, NW, RD], f16)
        outbuf = cpool.tile([P, NW * TPW], f32)

        with tc.tile_pool(name="ypsum", bufs=2, space="PSUM") as ypool:
            for w in range(NW):
                y_ps = ypool.tile([P, 512], f32, tag="y")
                nc.tensor.matmul(
                    y_ps[:, 0:RD], lhsT=ifT_sb[:, w * P : (w + 1) * P], rhs=p2_sb[:]
                )
                nc.scalar.activation(ysb[:, w, :], y_ps[:, 0:RD], AF.Copy)

        with (
            tc.tile_pool(name="gather", bufs=3) as gpool,
            tc.tile_pool(name="work", bufs=2) as wpool,
            tc.tile_pool(name="psum_yv", bufs=2, space="PSUM") as zpool,
        ):
            for w in range(NW):
                us_f = gpool.tile([P, TPW, 2 * D], f16, tag="usf")
                for q in range(NQ):
                    gi = w * NQ + q
                    nc.gpsimd.dma_gather(
                        out_ap=us_f[:, q * GT : (q + 1) * GT, :],
                        in_ap=ufeat_d[UQ_BASE[q] : UQ_BASE[q] + UQ_LEN[q], :],
                        idxs_ap=idx_sb[:, gi * S16 : (gi + 1) * S16],
                        num_idxs=G,
                        num_idxs_reg=G,
                        elem_size=2 * D,
                        queue_num=(w + q) % 4,
                    )
                selt = gpool.tile([P, NQ * G], f16, tag="selt")
                nc.sync.dma_start(selt[:], selt_d[w * P : (w + 1) * P, :])
                scorew = wpool.tile([P, TPW, R], f32, tag="scw")
                b24 = wpool.tile([P, TPW, R, D], f16, tag="b24")
                for sub in range(-(-TPW // 4)):
                    t0 = sub * 4
                    nb = min(4, TPW - t0)
                    yv_ps = zpool.tile([P, 4, 512], f32, tag="yv")
                    for i in range(nb):
                        t = t0 + i
                        nc.tensor.matmul(
                            yv_ps[:, i, 0:RD],
                            lhsT=selt[:, t * P : (t + 1) * P],
                            rhs=ysb[:, w, :],
                        )
                    yv_h = wpool.tile([P, 4, RD], f16, tag="yvh")
                    nc.scalar.activation(
                        yv_h[:, 0:nb, :], yv_ps[:, 0:nb, 0:RD], AF.Copy
                    )
                    nc.vector.tensor_mul(
                        b24[:, t0 : t0 + nb, :, :],
                        us_f[:, t0 : t0 + nb, 0:D]
                        .rearrange("p t (o d) -> p t o d", o=1)
                        .to_broadcast([P, nb, R, D]),
                        yv_h[:, 0:nb, :].rearrange("p t (r d) -> p t r d", r=R),
                    )
                t32 = wpool.tile([P, TPW, R, 32], f16, tag="t32")
                nc.vector.tensor_add(t32[:], b24[:, :, :, 0:32], b24[:, :, :, 32:64])
                t16 = wpool.tile([P, TPW, R, 16], f16, tag="t16")
                nc.vector.tensor_add(t16[:], t32[:, :, :, 0:16], t32[:, :, :, 16:32])
                t8 = wpool.tile([P, TPW, R, 8], f16, tag="t8")
                nc.vector.tensor_add(t8[:], t16[:, :, :, 0:8], t16[:, :, :, 8:16])
                t4 = wpool.tile([P, TPW, R, 4], f32, tag="t4")
                nc.vector.tensor_add(t4[:], t8[:, :, :, 0:4], t8[:, :, :, 4:8])
                nc.vector.tensor_reduce(
                    out=scorew[:], in_=t4[:], axis=AX.X, op=ALU.add
                )
                e_t = wpool.tile([P, TPW * R], f32, tag="et")
                nc.scalar.activation(
                    e_t[:], scorew[:].rearrange("p t r -> p (t r)"), AF.Exp
                )
                den = wpool.tile([P, TPW], f32, tag="den")
                nc.vector.tensor_reduce(
                    out=den[:],
                    in_=e_t[:].rearrange("p (t r) -> p t r", r=R),
                    axis=AX.X,
                    op=ALU.add,
                )
                nums = wpool.tile([P, TPW * R], f32, tag="nums")
                vals_bc = (
                    vals_sb[:]
                    .rearrange("p (o r) -> p o r", o=1)
                    .to_broadcast([P, TPW, R])
                )
                nc.vector.tensor_mul(
                    nums[:].rearrange("p (t r) -> p t r", r=R),
                    e_t[:].rearrange("p (t r) -> p t r", r=R),
                    vals_bc,
                )
                num = wpool.tile([P, TPW], f32, tag="num")
                nc.vector.tensor_reduce(
                    out=num[:],
                    in_=nums[:].rearrange("p (t r) -> p t r", r=R),
                    axis=AX.X,
                    op=ALU.add,
                )
                rden = wpool.tile([P, TPW], f32, tag="rden")
                nc.vector.reciprocal(rden[:], den[:])
                nc.vector.tensor_mul(
                    outbuf[:, w * TPW : (w + 1) * TPW], num[:], rden[:]
                )
            nc.sync.dma_start(out_d[:], outbuf[:])


def _build(NW, G):
    import concourse.bacc as bacc
    import concourse.mybir as mybir
    import concourse.tile as tile

    nc = bacc.Bacc(None, target_bir_lowering=False, num_swdge_queues=4)
    f32, f16, i16 = mybir.dt.float32, mybir.dt.float16, mybir.dt.int16
    TPW = NQ * G // P
    NG = NW * NQ
    ufeat_d = nc.dram_tensor("ufeat", [N_USERS, 2 * D], f16, kind="ExternalInput")
    ifT_d = nc.dram_tensor("ifT", [D, NW * P], f16, kind="ExternalInput")
    p2_d = nc.dram_tensor("p2", [D, RD], f16, kind="ExternalInput")
    idxu_d = nc.dram_tensor("idxu", [P, NG * (G // 16)], i16, kind="ExternalInput")
    selt_d = nc.dram_tensor("selt", [NW * P, NQ * G], f16, kind="ExternalInput")
    vals_d = nc.dram_tensor("vals", [P, R], f32, kind="ExternalInput")
    out_d = nc.dram_tensor("out", [P, NW * TPW], f32, kind="ExternalOutput")

    with tile.TileContext(nc) as tc:
        _emit(
            nc,
            tc,
            (ufeat_d, ifT_d, p2_d, idxu_d, selt_d, vals_d, out_d),
            NW,
            G,
        )
    nc.compile()
    return nc


def _install_profile_hook():
    """Make antenv.axon_hooks available so run_bass_kernel_spmd(trace=True)
    can capture NTFF profiles through the axon .so (used by test.py only)."""
    import types

    try:
        from antenv.axon_hooks import get_axon_ntff_profile_hook  # noqa: F401

        return
    except ImportError:
        pass
    import antenv
    from trn_agent_boot.trn_boot import _ntff_profile_via_ctypes

    hook = _ntff_profile_via_ctypes("/opt/axon/libaxon_pjrt.so")
    mod = types.ModuleType("antenv.axon_hooks")
    mod._hook = hook
    mod.get_axon_ntff_profile_hook = lambda: mod._hook
    mod.set_axon_ntff_profile_hook = lambda h: setattr(mod, "_hook", h)
    sys.modules["antenv.axon_hooks"] = mod
    antenv.axon_hooks = mod


def kernel(ufeat, ifeat, Ps, src, dst):
    from concourse.bass_utils import run_bass_kernel_spmd

    ufeat = np.asarray(ufeat, np.float32)
    ifeat = np.asarray(ifeat, np.float32)
    Ps = np.asarray(Ps, np.float32)
    src = np.asarray(src, np.int32)
    dst = np.asarray(dst, np.int32)

    in_maps, metas = _prepare(ufeat, ifeat, Ps, src, dst)
    NW, G = _NC_CACHE["params"]
    key = ("nc", NW, G)
    if key not in _NC_CACHE:
        _NC_CACHE[key] = _build(NW, G)
        _NC_CACHE["nc"] = _NC_CACHE[key]
    nc = _NC_CACHE[key]
    res = run_bass_kernel_spmd(nc, in_maps, core_ids=list(range(N_CORES)))
    out = np.zeros(E, np.float32)
    for c in range(N_CORES):
        o = res.results[c]["out"]  # [P, NW*TPW]
        flat = o.T.reshape(-1)  # slot-ordered
        s2e = metas[c]
        valid = s2e >= 0
        out[s2e[valid]] = flat[valid]
    return out


# revision 18
# speedup vs baseline: 1.1983x; 1.1983x over previous
"""TRN2 Bass kernel for nn_BiDecoder (GNN edge rating decoder), 8 NeuronCores.

ratings[e] = sum_r softmax_r(ufeat[src[e]] @ Ps[r] @ ifeat[dst[e]]) * (r+1)

v2 design:
  - Edges dst-sorted into 8 contiguous shards (each core owns an item band).
  - Within a core, items are covered by NW aligned 128-item windows; edges are
    grouped by (window w, src-quarter q) and sorted by dst inside each group.
    Each group gets G slots (G multiple of 256); trailing pads idx=-1.
  - Item side needs NO gather: Y[j, (r,d)] = sum_f Ps[r,d,f] ifeat[j,f] is
    precomputed per window on PE (fp16); per tile a one-hot SelT (built with
    tensor_mask_reduce run-intervals, since dst-sorted edges give each item a
    contiguous slot run) expands Y rows to edges via one 320-col matmul.
  - User side: dma_gather of ufeat rows, 4 src-quarters (int16 idx < 25000)
    issued round-robin on 4 SWDGE queues so descriptor generation runs on all
    Q7 cpu pairs in parallel.
  - scores = sum_d us*yv via fp16 DVE mul (2x) + binary-tree adds (2x),
    softmax (no max-sub; scores bounded) -> ratings.
"""
import sys

sys.path.insert(0, "/opt/trn_rl_repo")
import numpy as np

P = 128
D = 64
R = 5
RD = R * D
N_USERS, N_ITEMS, E = 100000, 50000, 1000000
N_CORES = 8
E_CORE = E // N_CORES
NQ = 5  # overlapping int16-addressable src groups
UQ_BASE = (0, 16808, 33616, 50424, 67232)
UQ_LEN = (32768, 32768, 32768, 32768, 32768)
C_TARGET = 2500

_NC_CACHE = {}


def _plan(src, dst):
    """Shard edges; greedy equal-count windows (item span <= 128, cut mid-item
    is fine since SelT is host-built); balanced src-quarters with overlapping
    int16-addressable ranges. Returns per-core layout + global (NW, G)."""
    perm = np.argsort(dst, kind="stable")
    shards = perm.reshape(N_CORES, E_CORE)
    A = np.array(UQ_BASE)
    cores = []
    maxw = 0
    maxg = 0
    for c in range(N_CORES):
        eid = shards[c]
        d = dst[eid].astype(np.int64)
        s = src[eid].astype(np.int64)
        it0s = []
        bounds = [0]
        i = 0
        while i < E_CORE:
            it0 = int(d[i])
            j = min(
                i + C_TARGET,
                E_CORE,
                int(np.searchsorted(d, it0 + P, side="left")),
            )
            it0s.append(it0)
            bounds.append(j)
            i = j
        nw = len(it0s)
        w = np.zeros(E_CORE, np.int64)
        for wi in range(nw):
            w[bounds[wi] : bounds[wi + 1]] = wi
        it0_arr = np.array(it0s, np.int64)
        q = np.zeros(E_CORE, np.int64)
        for wi in range(nw):
            lo, hi = bounds[wi], bounds[wi + 1]
            ss = s[lo:hi]
            n = hi - lo
            q_hi = np.searchsorted(A, ss, side="right") - 1
            can_lo = (q_hi > 0) & (ss <= A[np.maximum(q_hi - 1, 0)] + 32767)
            forced = np.bincount(q_hi[~can_lo], minlength=NQ)
            flexz = np.bincount(q_hi[can_lo] - 1, minlength=NQ)[: NQ - 1]
            T = -(-n // NQ)
            qq = q_hi.copy()
            inbound = 0
            for z in range(NQ - 1):
                x = int(np.clip(T - forced[z] - inbound, 0, flexz[z]))
                idxz = np.where(can_lo & (q_hi - 1 == z))[0]
                qq[idxz[:x]] = z
                inbound = flexz[z] - x
            q[lo:hi] = qq
        g = w * NQ + q
        order = np.argsort(g, kind="stable")
        eid, d, s, w, q, g = (a[order] for a in (eid, d, s, w, q, g))
        cnt = np.bincount(g, minlength=nw * NQ)
        maxg = max(maxg, int(cnt.max()))
        maxw = max(maxw, nw)
        cores.append((eid, d, s, w, q, g, it0_arr, nw, cnt))
    NW = maxw
    G = max(128, -(-maxg // 128) * 128)
    return cores, NW, G


def _prepare(ufeat, ifeat, Ps, src, dst):
    ufeat = np.asarray(ufeat, np.float32)
    ifeat = np.asarray(ifeat, np.float32)
    Ps = np.asarray(Ps, np.float32)
    src = np.asarray(src)
    dst = np.asarray(dst)
    cores, NW, G = _plan(src, dst)
    _NC_CACHE["params"] = (NW, G)
    S16 = G // 16
    NG = NW * NQ
    p2 = np.ascontiguousarray(Ps.transpose(2, 0, 1).reshape(D, RD)).astype(np.float16)
    vals = np.tile(np.arange(1.0, 6.0, dtype=np.float32), (P, 1))
    uf = np.zeros((N_USERS, 2 * D), np.float16)
    uf[:, :D] = ufeat.astype(np.float16)
    in_maps, metas = [], []
    A = np.array(UQ_BASE)
    for (eid, d, s, w, q, g, it0_arr, nw, cnt) in cores:
        nslot = NG * G
        starts = np.zeros(NG + 1, np.int64)
        starts[1 : len(cnt) + 1] = np.cumsum(cnt)
        pos = np.arange(E_CORE) - starts[g]
        slot = g * G + pos
        idxu = np.zeros(nslot, np.int16)
        idxu[slot] = (s - A[q]).astype(np.int16)
        slot2eid = np.full(nslot, -1, np.int64)
        slot2eid[slot] = eid
        dl = d - it0_arr[w]  # 0..127 within window slab
        assert dl.min() >= 0 and dl.max() < P
        # host-built one-hot SelT: [row j, slot] = 1 iff slot's dst-local == j
        selt_h = np.zeros((P, nslot), np.float16)
        selt_h[dl, slot] = 1.0
        wrapped = (
            idxu.reshape(NG, S16, 16).transpose(2, 0, 1).reshape(16, NG * S16)
        )  # [16, (group-major, col-minor)]; element i of group = [i%16, i//16]
        wrapped = np.tile(wrapped, (8, 1)).astype(np.int16)
        slabs = np.zeros((NW * P, D), np.float32)
        for wi in range(nw):
            it0 = int(it0_arr[wi])
            navail = min(P, N_ITEMS - it0)
            slabs[wi * P : wi * P + navail] = ifeat[it0 : it0 + navail]
        ifT = np.ascontiguousarray(slabs.T).astype(np.float16)
        in_maps.append(
            {
                "ufeat": uf,
                "ifT": ifT,
                "p2": p2,
                "idxu": wrapped,
                "selt": np.ascontiguousarray(
                    selt_h.reshape(P, NW, NQ * G).transpose(1, 0, 2).reshape(
                        NW * P, NQ * G
                    )
                ),
                "vals": vals,
            }
        )
        metas.append(slot2eid)
    return in_maps, metas


def _emit(nc, tc, aps, NW, G):
    import concourse.mybir as mybir
    from concourse import library_config

    f32, f16, i16 = mybir.dt.float32, mybir.dt.float16, mybir.dt.int16
    AF = mybir.ActivationFunctionType
    ALU = mybir.AluOpType
    AX = mybir.AxisListType
    TPW = NQ * G // P
    NG = NW * NQ
    S16 = G // 16
    GT = G // P  # tiles per group
    ufeat_d, ifT_d, p2_d, idxu_d, selt_d, vals_d, out_d = aps

    nc.gpsimd.load_library(library_config.mlp)
    with tc.tile_pool(name="const", bufs=1) as cpool:
        p2_sb = cpool.tile([D, RD], f16)
        nc.sync.dma_start(p2_sb[:], p2_d[:])
        ifT_sb = cpool.tile([D, NW * P], f16)
        nc.sync.dma_start(ifT_sb[:], ifT_d[:])
        idx_sb = cpool.tile([P, NG * S16], i16)
        nc.sync.dma_start(idx_sb[:], idxu_d[:])
        vals_sb = cpool.tile([P, R], f32)
        nc.sync.dma_start(vals_sb[:], vals_d[:])
        ysb = cpool.tile([P, NW, RD], f16)
        outbuf = cpool.tile([P, NW * TPW], f32)

        with tc.tile_pool(name="ypsum", bufs=2, space="PSUM") as ypool:
            for w in range(NW):
                y_ps = ypool.tile([P, 512], f32, tag="y")
                nc.tensor.matmul(
                    y_ps[:, 0:RD], lhsT=ifT_sb[:, w * P : (w + 1) * P], rhs=p2_sb[:]
                )
                nc.scalar.activation(ysb[:, w, :], y_ps[:, 0:RD], AF.Copy)

        with (
            tc.tile_pool(name="gather", bufs=3) as gpool,
            tc.tile_pool(name="work", bufs=2) as wpool,
            tc.tile_pool(name="psum_yv", bufs=2, space="PSUM") as zpool,
        ):
            for w in range(NW):
                us_f = gpool.tile([P, TPW, 2 * D], f16, tag="usf")
                for q in range(NQ):
                    gi = w * NQ + q
                    nc.gpsimd.dma_gather(
                        out_ap=us_f[:, q * GT : (q + 1) * GT, :],
                        in_ap=ufeat_d[UQ_BASE[q] : UQ_BASE[q] + UQ_LEN[q], :],
                        idxs_ap=idx_sb[:, gi * S16 : (gi + 1) * S16],
                        num_idxs=G,
                        num_idxs_reg=G,
                        elem_size=2 * D,
                        queue_num=(w * NQ + q) % 4,
                    )
                selt = gpool.tile([P, NQ * G], f16, tag="selt")
                nc.sync.dma_start(selt[:], selt_d[w * P : (w + 1) * P, :])
                scorew = wpool.tile([P, TPW, R], f32, tag="scw")
                b24 = wpool.tile([P, TPW, R, D], f16, tag="b24")
                for sub in range(-(-TPW // 4)):
                    t0 = sub * 4
                    nb = min(4, TPW - t0)
                    yv_ps = zpool.tile([P, 4, 512], f32, tag="yv")
                    for i in range(nb):
                        t = t0 + i
                        nc.tensor.matmul(
                            yv_ps[:, i, 0:RD],
                            lhsT=selt[:, t * P : (t + 1) * P],
                            rhs=ysb[:, w, :],
                        )
                    yv_h = wpool.tile([P, 4, RD], f16, tag="yvh")
                    nc.scalar.activation(
                        yv_h[:, 0:nb, :], yv_ps[:, 0:nb, 0:RD], AF.Copy
                    )
                    nc.vector.tensor_mul(
                        b24[:, t0 : t0 + nb, :, :],
                        us_f[:, t0 : t0 + nb, 0:D]
                        .rearrange("p t (o d) -> p t o d", o=1)
                        .to_broadcast([P, nb, R, D]),
                        yv_h[:, 0:nb, :].rearrange("p t (r d) -> p t r d", r=R),
                    )
                t32 = wpool.tile([P, TPW, R, 32], f16, tag="t32")
                nc.vector.tensor_add(t32[:], b24[:, :, :, 0:32], b24[:, :, :, 32:64])
                t16 = wpool.tile([P, TPW, R, 16], f16, tag="t16")
                nc.vector.tensor_add(t16[:], t32[:, :, :, 0:16], t32[:, :, :, 16:32])
                t8 = wpool.tile([P, TPW, R, 8], f16, tag="t8")
                nc.vector.tensor_add(t8[:], t16[:, :, :, 0:8], t16[:, :, :, 8:16])
                t4 = wpool.tile([P, TPW, R, 4], f32, tag="t4")
                nc.vector.tensor_add(t4[:], t8[:, :, :, 0:4], t8[:, :, :, 4:8])
                nc.vector.tensor_reduce(
                    out=scorew[:], in_=t4[:], axis=AX.X, op=ALU.add
                )
                e_t = wpool.tile([P, TPW * R], f32, tag="et")
                nc.scalar.activation(
                    e_t[:], scorew[:].rearrange("p t r -> p (t r)"), AF.Exp
                )
                den = wpool.tile([P, TPW], f32, tag="den")
                nc.vector.tensor_reduce(
                    out=den[:],
                    in_=e_t[:].rearrange("p (t r) -> p t r", r=R),
                    axis=AX.X,
                    op=ALU.add,
                )
                nums = wpool.tile([P, TPW * R], f32, tag="nums")
                vals_bc = (
                    vals_sb[:]
                    .rearrange("p (o r) -> p o r", o=1)
                    .to_broadcast([P, TPW, R])
                )
                nc.vector.tensor_mul(
                    nums[:].rearrange("p (t r) -> p t r", r=R),
                    e_t[:].rearrange("p (t r) -> p t r", r=R),
                    vals_bc,
                )
                num = wpool.tile([P, TPW], f32, tag="num")
                nc.vector.tensor_reduce(
                    out=num[:],
                    in_=nums[:].rearrange("p (t r) -> p t r", r=R),
                    axis=AX.X,
                    op=ALU.add,
                )
                rden = wpool.tile([P, TPW], f32, tag="rden")
                nc.vector.reciprocal(rden[:], den[:])
                nc.vector.tensor_mul(
                    outbuf[:, w * TPW : (w + 1) * TPW], num[:], rden[:]
                )
            nc.sync.dma_start(out_d[:], outbuf[:])


def _build(NW, G):
    import concourse.bacc as bacc
    import concourse.mybir as mybir
    import concourse.tile as tile

    nc = bacc.Bacc(None, target_bir_lowering=False, num_swdge_queues=4)
    f32, f16, i16 = mybir.dt.float32, mybir.dt.float16, mybir.dt.int16
    TPW = NQ * G // P
    NG = NW * NQ
    ufeat_d = nc.dram_tensor("ufeat", [N_USERS, 2 * D], f16, kind="ExternalInput")
    ifT_d = nc.dram_tensor("ifT", [D, NW * P], f16, kind="ExternalInput")
    p2_d = nc.dram_tensor("p2", [D, RD], f16, kind="ExternalInput")
    idxu_d = nc.dram_tensor("idxu", [P, NG * (G // 16)], i16, kind="ExternalInput")
    selt_d = nc.dram_tensor("selt", [NW * P, NQ * G], f16, kind="ExternalInput")
    vals_d = nc.dram_tensor("vals", [P, R], f32, kind="ExternalInput")
    out_d = nc.dram_tensor("out", [P, NW * TPW], f32, kind="ExternalOutput")

    with tile.TileContext(nc) as tc:
        _emit(
            nc,
            tc,
            (ufeat_d, ifT_d, p2_d, idxu_d, selt_d, vals_d, out_d),
            NW,
            G,
        )
    nc.compile()
    return nc


def _install_profile_hook():
    """Make antenv.axon_hooks available so run_bass_kernel_spmd(trace=True)
    can capture NTFF profiles through the axon .so (used by test.py only)."""
    import types

    try:
        from antenv.axon_hooks import get_axon_ntff_profile_hook  # noqa: F401

        return
    except ImportError:
        pass
    import antenv
    from trn_agent_boot.trn_boot import _ntff_profile_via_ctypes

    hook = _ntff_profile_via_ctypes("/opt/axon/libaxon_pjrt.so")
    mod = types.ModuleType("antenv.axon_hooks")
    mod._hook = hook
    mod.get_axon_ntff_profile_hook = lambda: mod._hook
    mod.set_axon_ntff_profile_hook = lambda h: setattr(mod, "_hook", h)
    sys.modules["antenv.axon_hooks"] = mod
    antenv.axon_hooks = mod


def kernel(ufeat, ifeat, Ps, src, dst):
    from concourse.bass_utils import run_bass_kernel_spmd

    ufeat = np.asarray(ufeat, np.float32)
    ifeat = np.asarray(ifeat, np.float32)
    Ps = np.asarray(Ps, np.float32)
    src = np.asarray(src, np.int32)
    dst = np.asarray(dst, np.int32)

    in_maps, metas = _prepare(ufeat, ifeat, Ps, src, dst)
    NW, G = _NC_CACHE["params"]
    key = ("nc", NW, G)
    if key not in _NC_CACHE:
        _NC_CACHE[key] = _build(NW, G)
        _NC_CACHE["nc"] = _NC_CACHE[key]
    nc = _NC_CACHE[key]
    res = run_bass_kernel_spmd(nc, in_maps, core_ids=list(range(N_CORES)))
    out = np.zeros(E, np.float32)
    for c in range(N_CORES):
        o = res.results[c]["out"]  # [P, NW*TPW]
        flat = o.T.reshape(-1)  # slot-ordered
        s2e = metas[c]
        valid = s2e >= 0
        out[s2e[valid]] = flat[valid]
    return out


# revision 19
# speedup vs baseline: 1.2697x; 1.0597x over previous
"""TRN2 Bass kernel for nn_BiDecoder (GNN edge rating decoder), 8 NeuronCores.

ratings[e] = sum_r softmax_r(ufeat[src[e]] @ Ps[r] @ ifeat[dst[e]]) * (r+1)

v2 design:
  - Edges dst-sorted into 8 contiguous shards (each core owns an item band).
  - Within a core, items are covered by NW aligned 128-item windows; edges are
    grouped by (window w, src-quarter q) and sorted by dst inside each group.
    Each group gets G slots (G multiple of 256); trailing pads idx=-1.
  - Item side needs NO gather: Y[j, (r,d)] = sum_f Ps[r,d,f] ifeat[j,f] is
    precomputed per window on PE (fp16); per tile a one-hot SelT (built with
    tensor_mask_reduce run-intervals, since dst-sorted edges give each item a
    contiguous slot run) expands Y rows to edges via one 320-col matmul.
  - User side: dma_gather of ufeat rows, 4 src-quarters (int16 idx < 25000)
    issued round-robin on 4 SWDGE queues so descriptor generation runs on all
    Q7 cpu pairs in parallel.
  - scores = sum_d us*yv via fp16 DVE mul (2x) + binary-tree adds (2x),
    softmax (no max-sub; scores bounded) -> ratings.
"""
import sys

sys.path.insert(0, "/opt/trn_rl_repo")
import numpy as np

P = 128
D = 64
R = 5
RD = R * D
N_USERS, N_ITEMS, E = 100000, 50000, 1000000
N_CORES = 8
E_CORE = E // N_CORES
NQ = 5  # overlapping int16-addressable src groups
UQ_BASE = (0, 16808, 33616, 50424, 67232)
UQ_LEN = (32768, 32768, 32768, 32768, 32768)
C_TARGET = 2500

_NC_CACHE = {}


def _plan(src, dst):
    """Shard edges; greedy equal-count windows (item span <= 128, cut mid-item
    is fine since SelT is host-built); balanced src-quarters with overlapping
    int16-addressable ranges. Returns per-core layout + global (NW, G)."""
    perm = np.argsort(dst, kind="stable")
    shards = perm.reshape(N_CORES, E_CORE)
    A = np.array(UQ_BASE)
    cores = []
    maxw = 0
    maxg = 0
    for c in range(N_CORES):
        eid = shards[c]
        d = dst[eid].astype(np.int64)
        s = src[eid].astype(np.int64)
        it0s = []
        bounds = [0]
        i = 0
        while i < E_CORE:
            it0 = int(d[i])
            j = min(
                i + C_TARGET,
                E_CORE,
                int(np.searchsorted(d, it0 + P, side="left")),
            )
            it0s.append(it0)
            bounds.append(j)
            i = j
        nw = len(it0s)
        w = np.zeros(E_CORE, np.int64)
        for wi in range(nw):
            w[bounds[wi] : bounds[wi + 1]] = wi
        it0_arr = np.array(it0s, np.int64)
        q = np.zeros(E_CORE, np.int64)
        for wi in range(nw):
            lo, hi = bounds[wi], bounds[wi + 1]
            ss = s[lo:hi]
            n = hi - lo
            q_hi = np.searchsorted(A, ss, side="right") - 1
            can_lo = (q_hi > 0) & (ss <= A[np.maximum(q_hi - 1, 0)] + 32767)
            forced = np.bincount(q_hi[~can_lo], minlength=NQ)
            flexz = np.bincount(q_hi[can_lo] - 1, minlength=NQ)[: NQ - 1]
            T = -(-n // NQ)
            qq = q_hi.copy()
            inbound = 0
            for z in range(NQ - 1):
                x = int(np.clip(T - forced[z] - inbound, 0, flexz[z]))
                idxz = np.where(can_lo & (q_hi - 1 == z))[0]
                qq[idxz[:x]] = z
                inbound = flexz[z] - x
            q[lo:hi] = qq
        g = w * NQ + q
        order = np.argsort(g, kind="stable")
        eid, d, s, w, q, g = (a[order] for a in (eid, d, s, w, q, g))
        cnt = np.bincount(g, minlength=nw * NQ)
        maxg = max(maxg, int(cnt.max()))
        maxw = max(maxw, nw)
        cores.append((eid, d, s, w, q, g, it0_arr, nw, cnt))
    NW = maxw
    G = max(128, -(-maxg // 128) * 128)
    return cores, NW, G


def _prepare(ufeat, ifeat, Ps, src, dst):
    ufeat = np.asarray(ufeat, np.float32)
    ifeat = np.asarray(ifeat, np.float32)
    Ps = np.asarray(Ps, np.float32)
    src = np.asarray(src)
    dst = np.asarray(dst)
    cores, NW, G = _plan(src, dst)
    _NC_CACHE["params"] = (NW, G)
    S16 = G // 16
    NG = NW * NQ
    p2 = np.ascontiguousarray(Ps.transpose(2, 0, 1).reshape(D, RD)).astype(np.float16)
    vals = np.tile(np.arange(1.0, 6.0, dtype=np.float32), (P, 1))
    uf = np.zeros((N_USERS, 2 * D), np.float16)
    uf[:, :D] = ufeat.astype(np.float16)
    in_maps, metas = [], []
    A = np.array(UQ_BASE)
    for (eid, d, s, w, q, g, it0_arr, nw, cnt) in cores:
        nslot = NG * G
        starts = np.zeros(NG + 1, np.int64)
        starts[1 : len(cnt) + 1] = np.cumsum(cnt)
        pos = np.arange(E_CORE) - starts[g]
        slot = g * G + pos
        idxu = np.zeros(nslot, np.int16)
        idxu[slot] = (s - A[q]).astype(np.int16)
        slot2eid = np.full(nslot, -1, np.int64)
        slot2eid[slot] = eid
        dl = d - it0_arr[w]  # 0..127 within window slab
        assert dl.min() >= 0 and dl.max() < P
        # host-built one-hot SelT: [row j, slot] = 1 iff slot's dst-local == j
        selt_h = np.zeros((P, nslot), np.float16)
        selt_h[dl, slot] = 1.0
        wrapped = (
            idxu.reshape(NG, S16, 16).transpose(2, 0, 1).reshape(16, NG * S16)
        )  # [16, (group-major, col-minor)]; element i of group = [i%16, i//16]
        wrapped = np.tile(wrapped, (8, 1)).astype(np.int16)
        slabs = np.zeros((NW * P, D), np.float32)
        for wi in range(nw):
            it0 = int(it0_arr[wi])
            navail = min(P, N_ITEMS - it0)
            slabs[wi * P : wi * P + navail] = ifeat[it0 : it0 + navail]
        ifT = np.ascontiguousarray(slabs.T).astype(np.float16)
        in_maps.append(
            {
                "ufeat": uf,
                "ifT": ifT,
                "p2": p2,
                "idxu": wrapped,
                "selt": np.ascontiguousarray(
                    selt_h.reshape(P, NW, NQ * G).transpose(1, 0, 2).reshape(
                        NW * P, NQ * G
                    )
                ),
                "vals": vals,
            }
        )
        metas.append(slot2eid)
    return in_maps, metas


def _emit(nc, tc, aps, NW, G):
    import concourse.mybir as mybir
    from concourse import library_config

    f32, f16, i16 = mybir.dt.float32, mybir.dt.float16, mybir.dt.int16
    AF = mybir.ActivationFunctionType
    ALU = mybir.AluOpType
    AX = mybir.AxisListType
    TPW = NQ * G // P
    NG = NW * NQ
    S16 = G // 16
    GT = G // P  # tiles per group
    ufeat_d, ifT_d, p2_d, idxu_d, selt_d, vals_d, out_d = aps

    nc.gpsimd.load_library(library_config.mlp)
    with tc.tile_pool(name="const", bufs=1) as cpool:
        p2_sb = cpool.tile([D, RD], f16)
        nc.sync.dma_start(p2_sb[:], p2_d[:])
        ifT_sb = cpool.tile([D, NW * P], f16)
        nc.sync.dma_start(ifT_sb[:], ifT_d[:])
        idx_sb = cpool.tile([P, NG * S16], i16)
        nc.sync.dma_start(idx_sb[:], idxu_d[:])
        vals_sb = cpool.tile([P, R], f32)
        nc.sync.dma_start(vals_sb[:], vals_d[:])
        ysb = cpool.tile([P, NW, RD], f16)
        outbuf = cpool.tile([P, NW * TPW], f32)

        with tc.tile_pool(name="ypsum", bufs=2, space="PSUM") as ypool:
            for w in range(NW):
                y_ps = ypool.tile([P, 512], f32, tag="y")
                nc.tensor.matmul(
                    y_ps[:, 0:RD], lhsT=ifT_sb[:, w * P : (w + 1) * P], rhs=p2_sb[:]
                )
                nc.scalar.activation(ysb[:, w, :], y_ps[:, 0:RD], AF.Copy)

        with (
            tc.tile_pool(name="gather", bufs=4) as gpool,
            tc.tile_pool(name="work", bufs=2) as wpool,
            tc.tile_pool(name="psum_yv", bufs=2, space="PSUM") as zpool,
        ):
            for w in range(NW):
                us_f = gpool.tile([P, TPW, 2 * D], f16, tag="usf")
                for q in range(NQ):
                    gi = w * NQ + q
                    nc.gpsimd.dma_gather(
                        out_ap=us_f[:, q * GT : (q + 1) * GT, :],
                        in_ap=ufeat_d[UQ_BASE[q] : UQ_BASE[q] + UQ_LEN[q], :],
                        idxs_ap=idx_sb[:, gi * S16 : (gi + 1) * S16],
                        num_idxs=G,
                        num_idxs_reg=G,
                        elem_size=2 * D,
                        queue_num=(w * NQ + q) % 4,
                    )
                selt = gpool.tile([P, NQ * G], f16, tag="selt")
                nc.sync.dma_start(selt[:], selt_d[w * P : (w + 1) * P, :])
                scorew = wpool.tile([P, TPW, R], f32, tag="scw")
                b24 = wpool.tile([P, TPW, R, D], f16, tag="b24")
                nsub = -(-TPW // 4)
                yv_h = None
                for sub in range(nsub):
                    t0 = sub * 4
                    nb = min(4, TPW - t0)
                    yv_ps = zpool.tile([P, 4, 512], f32, tag="yv")
                    for i in range(nb):
                        t = t0 + i
                        nc.tensor.matmul(
                            yv_ps[:, i, 0:RD],
                            lhsT=selt[:, t * P : (t + 1) * P],
                            rhs=ysb[:, w, :],
                        )
                    half = sub % 2
                    if half == 0:
                        yv_h = wpool.tile([P, 8, RD], f16, tag="yvh")
                    nc.scalar.activation(
                        yv_h[:, half * 4 : half * 4 + nb, :],
                        yv_ps[:, 0:nb, 0:RD],
                        AF.Copy,
                    )
                    if half == 1 or sub == nsub - 1:
                        m0 = (sub // 2) * 8
                        mb = t0 + nb - m0
                        nc.vector.tensor_mul(
                            b24[:, m0 : m0 + mb, :, :],
                            us_f[:, m0 : m0 + mb, 0:D]
                            .rearrange("p t (o d) -> p t o d", o=1)
                            .to_broadcast([P, mb, R, D]),
                            yv_h[:, 0:mb, :].rearrange(
                                "p t (r d) -> p t r d", r=R
                            ),
                        )
                t32 = wpool.tile([P, TPW, R, 32], f16, tag="t32")
                nc.vector.tensor_add(t32[:], b24[:, :, :, 0:32], b24[:, :, :, 32:64])
                t16 = wpool.tile([P, TPW, R, 16], f16, tag="t16")
                nc.vector.tensor_add(t16[:], t32[:, :, :, 0:16], t32[:, :, :, 16:32])
                t8 = wpool.tile([P, TPW, R, 8], f16, tag="t8")
                nc.vector.tensor_add(t8[:], t16[:, :, :, 0:8], t16[:, :, :, 8:16])
                t4 = wpool.tile([P, TPW, R, 4], f16, tag="t4")
                nc.vector.tensor_add(t4[:], t8[:, :, :, 0:4], t8[:, :, :, 4:8])
                nc.vector.tensor_reduce(
                    out=scorew[:], in_=t4[:], axis=AX.X, op=ALU.add
                )
                e_t = wpool.tile([P, TPW * R], f32, tag="et")
                nc.scalar.activation(
                    e_t[:], scorew[:].rearrange("p t r -> p (t r)"), AF.Exp
                )
                den = wpool.tile([P, TPW], f32, tag="den")
                nc.vector.tensor_reduce(
                    out=den[:],
                    in_=e_t[:].rearrange("p (t r) -> p t r", r=R),
                    axis=AX.X,
                    op=ALU.add,
                )
                nums = wpool.tile([P, TPW * R], f32, tag="nums")
                vals_bc = (
                    vals_sb[:]
                    .rearrange("p (o r) -> p o r", o=1)
                    .to_broadcast([P, TPW, R])
                )
                nc.vector.tensor_mul(
                    nums[:].rearrange("p (t r) -> p t r", r=R),
                    e_t[:].rearrange("p (t r) -> p t r", r=R),
                    vals_bc,
                )
                num = wpool.tile([P, TPW], f32, tag="num")
                nc.vector.tensor_reduce(
                    out=num[:],
                    in_=nums[:].rearrange("p (t r) -> p t r", r=R),
                    axis=AX.X,
                    op=ALU.add,
                )
                rden = wpool.tile([P, TPW], f32, tag="rden")
                nc.vector.reciprocal(rden[:], den[:])
                nc.vector.tensor_mul(
                    outbuf[:, w * TPW : (w + 1) * TPW], num[:], rden[:]
                )
            nc.sync.dma_start(out_d[:], outbuf[:])


def _build(NW, G):
    import concourse.bacc as bacc
    import concourse.mybir as mybir
    import concourse.tile as tile

    nc = bacc.Bacc(None, target_bir_lowering=False, num_swdge_queues=4)
    f32, f16, i16 = mybir.dt.float32, mybir.dt.float16, mybir.dt.int16
    TPW = NQ * G // P
    NG = NW * NQ
    ufeat_d = nc.dram_tensor("ufeat", [N_USERS, 2 * D], f16, kind="ExternalInput")
    ifT_d = nc.dram_tensor("ifT", [D, NW * P], f16, kind="ExternalInput")
    p2_d = nc.dram_tensor("p2", [D, RD], f16, kind="ExternalInput")
    idxu_d = nc.dram_tensor("idxu", [P, NG * (G // 16)], i16, kind="ExternalInput")
    selt_d = nc.dram_tensor("selt", [NW * P, NQ * G], f16, kind="ExternalInput")
    vals_d = nc.dram_tensor("vals", [P, R], f32, kind="ExternalInput")
    out_d = nc.dram_tensor("out", [P, NW * TPW], f32, kind="ExternalOutput")

    with tile.TileContext(nc) as tc:
        _emit(
            nc,
            tc,
            (ufeat_d, ifT_d, p2_d, idxu_d, selt_d, vals_d, out_d),
            NW,
            G,
        )
    nc.compile()
    return nc


def _install_profile_hook():
    """Make antenv.axon_hooks available so run_bass_kernel_spmd(trace=True)
    can capture NTFF profiles through the axon .so (used by test.py only)."""
    import types

    try:
        from antenv.axon_hooks import get_axon_ntff_profile_hook  # noqa: F401

        return
    except ImportError:
        pass
    import antenv
    from trn_agent_boot.trn_boot import _ntff_profile_via_ctypes

    hook = _ntff_profile_via_ctypes("/opt/axon/libaxon_pjrt.so")
    mod = types.ModuleType("antenv.axon_hooks")
    mod._hook = hook
    mod.get_axon_ntff_profile_hook = lambda: mod._hook
    mod.set_axon_ntff_profile_hook = lambda h: setattr(mod, "_hook", h)
    sys.modules["antenv.axon_hooks"] = mod
    antenv.axon_hooks = mod


def kernel(ufeat, ifeat, Ps, src, dst):
    from concourse.bass_utils import run_bass_kernel_spmd

    ufeat = np.asarray(ufeat, np.float32)
    ifeat = np.asarray(ifeat, np.float32)
    Ps = np.asarray(Ps, np.float32)
    src = np.asarray(src, np.int32)
    dst = np.asarray(dst, np.int32)

    in_maps, metas = _prepare(ufeat, ifeat, Ps, src, dst)
    NW, G = _NC_CACHE["params"]
    key = ("nc", NW, G)
    if key not in _NC_CACHE:
        _NC_CACHE[key] = _build(NW, G)
        _NC_CACHE["nc"] = _NC_CACHE[key]
    nc = _NC_CACHE[key]
    res = run_bass_kernel_spmd(nc, in_maps, core_ids=list(range(N_CORES)))
    out = np.zeros(E, np.float32)
    for c in range(N_CORES):
        o = res.results[c]["out"]  # [P, NW*TPW]
        flat = o.T.reshape(-1)  # slot-ordered
        s2e = metas[c]
        valid = s2e >= 0
        out[s2e[valid]] = flat[valid]
    return out


# revision 20
# speedup vs baseline: 1.3007x; 1.0244x over previous
"""TRN2 Bass kernel for nn_BiDecoder (GNN edge rating decoder), 8 NeuronCores.

ratings[e] = sum_r softmax_r(ufeat[src[e]] @ Ps[r] @ ifeat[dst[e]]) * (r+1)

v2 design:
  - Edges dst-sorted into 8 contiguous shards (each core owns an item band).
  - Within a core, items are covered by NW aligned 128-item windows; edges are
    grouped by (window w, src-quarter q) and sorted by dst inside each group.
    Each group gets G slots (G multiple of 256); trailing pads idx=-1.
  - Item side needs NO gather: Y[j, (r,d)] = sum_f Ps[r,d,f] ifeat[j,f] is
    precomputed per window on PE (fp16); per tile a one-hot SelT (built with
    tensor_mask_reduce run-intervals, since dst-sorted edges give each item a
    contiguous slot run) expands Y rows to edges via one 320-col matmul.
  - User side: dma_gather of ufeat rows, 4 src-quarters (int16 idx < 25000)
    issued round-robin on 4 SWDGE queues so descriptor generation runs on all
    Q7 cpu pairs in parallel.
  - scores = sum_d us*yv via fp16 DVE mul (2x) + binary-tree adds (2x),
    softmax (no max-sub; scores bounded) -> ratings.
"""
import sys

sys.path.insert(0, "/opt/trn_rl_repo")
import numpy as np

P = 128
D = 64
R = 5
RD = R * D
N_USERS, N_ITEMS, E = 100000, 50000, 1000000
N_CORES = 8
E_CORE = E // N_CORES
NQ = 5  # overlapping int16-addressable src groups
UQ_BASE = (0, 16808, 33616, 50424, 67232)
UQ_LEN = (32768, 32768, 32768, 32768, 32768)
C_TARGET = 2520

_NC_CACHE = {}


def _plan(src, dst):
    """Shard edges; greedy equal-count windows (item span <= 128, cut mid-item
    is fine since SelT is host-built); balanced src-quarters with overlapping
    int16-addressable ranges. Returns per-core layout + global (NW, G)."""
    perm = np.argsort(dst, kind="stable")
    shards = perm.reshape(N_CORES, E_CORE)
    A = np.array(UQ_BASE)
    cores = []
    maxw = 0
    maxg = 0
    for c in range(N_CORES):
        eid = shards[c]
        d = dst[eid].astype(np.int64)
        s = src[eid].astype(np.int64)
        it0s = []
        bounds = [0]
        i = 0
        while i < E_CORE:
            it0 = int(d[i])
            j = min(
                i + C_TARGET,
                E_CORE,
                int(np.searchsorted(d, it0 + P, side="left")),
            )
            it0s.append(it0)
            bounds.append(j)
            i = j
        nw = len(it0s)
        w = np.zeros(E_CORE, np.int64)
        for wi in range(nw):
            w[bounds[wi] : bounds[wi + 1]] = wi
        it0_arr = np.array(it0s, np.int64)
        q = np.zeros(E_CORE, np.int64)
        for wi in range(nw):
            lo, hi = bounds[wi], bounds[wi + 1]
            ss = s[lo:hi]
            n = hi - lo
            q_hi = np.searchsorted(A, ss, side="right") - 1
            can_lo = (q_hi > 0) & (ss <= A[np.maximum(q_hi - 1, 0)] + 32767)
            forced = np.bincount(q_hi[~can_lo], minlength=NQ)
            flexz = np.bincount(q_hi[can_lo] - 1, minlength=NQ)[: NQ - 1]
            T = -(-n // NQ)
            qq = q_hi.copy()
            inbound = 0
            for z in range(NQ - 1):
                x = int(np.clip(T - forced[z] - inbound, 0, flexz[z]))
                idxz = np.where(can_lo & (q_hi - 1 == z))[0]
                qq[idxz[:x]] = z
                inbound = flexz[z] - x
            q[lo:hi] = qq
        g = w * NQ + q
        order = np.argsort(g, kind="stable")
        eid, d, s, w, q, g = (a[order] for a in (eid, d, s, w, q, g))
        cnt = np.bincount(g, minlength=nw * NQ)
        maxg = max(maxg, int(cnt.max()))
        maxw = max(maxw, nw)
        cores.append((eid, d, s, w, q, g, it0_arr, nw, cnt))
    NW = maxw
    G = max(128, -(-maxg // 128) * 128)
    return cores, NW, G


def _prepare(ufeat, ifeat, Ps, src, dst):
    ufeat = np.asarray(ufeat, np.float32)
    ifeat = np.asarray(ifeat, np.float32)
    Ps = np.asarray(Ps, np.float32)
    src = np.asarray(src)
    dst = np.asarray(dst)
    cores, NW, G = _plan(src, dst)
    _NC_CACHE["params"] = (NW, G)
    S16 = G // 16
    NG = NW * NQ
    p2 = np.ascontiguousarray(Ps.transpose(2, 0, 1).reshape(D, RD)).astype(np.float16)
    vals = np.tile(np.arange(1.0, 6.0, dtype=np.float32), (P, 1))
    uf = np.zeros((N_USERS, 2 * D), np.float16)
    uf[:, :D] = ufeat.astype(np.float16)
    in_maps, metas = [], []
    A = np.array(UQ_BASE)
    for (eid, d, s, w, q, g, it0_arr, nw, cnt) in cores:
        nslot = NG * G
        starts = np.zeros(NG + 1, np.int64)
        starts[1 : len(cnt) + 1] = np.cumsum(cnt)
        pos = np.arange(E_CORE) - starts[g]
        slot = g * G + pos
        idxu = np.zeros(nslot, np.int16)
        idxu[slot] = (s - A[q]).astype(np.int16)
        slot2eid = np.full(nslot, -1, np.int64)
        slot2eid[slot] = eid
        dl = d - it0_arr[w]  # 0..127 within window slab
        assert dl.min() >= 0 and dl.max() < P
        # host-built one-hot SelT (fp8): [row j, slot] = 1 iff dst-local == j
        import ml_dtypes

        selt_h = np.zeros((P, nslot), ml_dtypes.float8_e4m3fn)
        selt_h[dl, slot] = 1.0
        wrapped = (
            idxu.reshape(NG, S16, 16).transpose(2, 0, 1).reshape(16, NG * S16)
        )  # [16, (group-major, col-minor)]; element i of group = [i%16, i//16]
        wrapped = np.tile(wrapped, (8, 1)).astype(np.int16)
        slabs = np.zeros((NW * P, D), np.float32)
        for wi in range(nw):
            it0 = int(it0_arr[wi])
            navail = min(P, N_ITEMS - it0)
            slabs[wi * P : wi * P + navail] = ifeat[it0 : it0 + navail]
        ifT = np.ascontiguousarray(slabs.T).astype(np.float16)
        in_maps.append(
            {
                "ufeat": uf,
                "ifT": ifT,
                "p2": p2,
                "idxu": wrapped,
                "selt": np.ascontiguousarray(
                    selt_h.reshape(P, NW, NQ * G).transpose(1, 0, 2).reshape(
                        NW * P, NQ * G
                    )
                ),
                "vals": vals,
            }
        )
        metas.append(slot2eid)
    return in_maps, metas


def _emit(nc, tc, aps, NW, G):
    import concourse.mybir as mybir
    from concourse import library_config

    f32, f16, i16 = mybir.dt.float32, mybir.dt.float16, mybir.dt.int16
    AF = mybir.ActivationFunctionType
    ALU = mybir.AluOpType
    AX = mybir.AxisListType
    TPW = NQ * G // P
    NG = NW * NQ
    S16 = G // 16
    GT = G // P  # tiles per group
    ufeat_d, ifT_d, p2_d, idxu_d, selt_d, vals_d, out_d = aps

    nc.gpsimd.load_library(library_config.mlp)
    with tc.tile_pool(name="const", bufs=1) as cpool:
        p2_sb = cpool.tile([D, RD], f16)
        nc.sync.dma_start(p2_sb[:], p2_d[:])
        ifT_sb = cpool.tile([D, NW * P], f16)
        nc.sync.dma_start(ifT_sb[:], ifT_d[:])
        idx_sb = cpool.tile([P, NG * S16], i16)
        nc.sync.dma_start(idx_sb[:], idxu_d[:])
        vals_sb = cpool.tile([P, R], f32)
        nc.sync.dma_start(vals_sb[:], vals_d[:])
        ysb = cpool.tile([P, NW, RD], f16)
        outbuf = cpool.tile([P, NW * TPW], f32)

        with tc.tile_pool(name="ypsum", bufs=2, space="PSUM") as ypool:
            for w in range(NW):
                y_ps = ypool.tile([P, 512], f32, tag="y")
                nc.tensor.matmul(
                    y_ps[:, 0:RD], lhsT=ifT_sb[:, w * P : (w + 1) * P], rhs=p2_sb[:]
                )
                nc.scalar.activation(ysb[:, w, :], y_ps[:, 0:RD], AF.Copy)

        with (
            tc.tile_pool(name="gather", bufs=4) as gpool,
            tc.tile_pool(name="work", bufs=2) as wpool,
            tc.tile_pool(name="psum_yv", bufs=2, space="PSUM") as zpool,
        ):
            for w in range(NW):
                us_f = gpool.tile([P, TPW, 2 * D], f16, tag="usf")
                for q in range(NQ):
                    gi = w * NQ + q
                    nc.gpsimd.dma_gather(
                        out_ap=us_f[:, q * GT : (q + 1) * GT, :],
                        in_ap=ufeat_d[UQ_BASE[q] : UQ_BASE[q] + UQ_LEN[q], :],
                        idxs_ap=idx_sb[:, gi * S16 : (gi + 1) * S16],
                        num_idxs=G,
                        num_idxs_reg=G,
                        elem_size=2 * D,
                        queue_num=(w * NQ + q) % 4,
                    )
                selt = gpool.tile([P, NQ * G], mybir.dt.float8e4, tag="selt")
                nc.sync.dma_start(selt[:], selt_d[w * P : (w + 1) * P, :])
                scorew = wpool.tile([P, TPW, R], f32, tag="scw")
                b24 = wpool.tile([P, TPW, R, D], f16, tag="b24")
                nsub = -(-TPW // 4)
                yv_h = None
                for sub in range(nsub):
                    t0 = sub * 4
                    nb = min(4, TPW - t0)
                    yv_ps = zpool.tile([P, 4, 512], f32, tag="yv")
                    for i in range(nb):
                        t = t0 + i
                        nc.tensor.matmul(
                            yv_ps[:, i, 0:RD],
                            lhsT=selt[:, t * P : (t + 1) * P],
                            rhs=ysb[:, w, :],
                        )
                    half = sub % 2
                    if half == 0:
                        yv_h = wpool.tile([P, 8, RD], f16, tag="yvh")
                    nc.scalar.activation(
                        yv_h[:, half * 4 : half * 4 + nb, :],
                        yv_ps[:, 0:nb, 0:RD],
                        AF.Copy,
                    )
                    if half == 1 or sub == nsub - 1:
                        m0 = (sub // 2) * 8
                        mb = t0 + nb - m0
                        nc.vector.tensor_mul(
                            b24[:, m0 : m0 + mb, :, :],
                            us_f[:, m0 : m0 + mb, 0:D]
                            .rearrange("p t (o d) -> p t o d", o=1)
                            .to_broadcast([P, mb, R, D]),
                            yv_h[:, 0:mb, :].rearrange(
                                "p t (r d) -> p t r d", r=R
                            ),
                        )
                t32 = wpool.tile([P, TPW, R, 32], f16, tag="t32")
                nc.vector.tensor_add(t32[:], b24[:, :, :, 0:32], b24[:, :, :, 32:64])
                t16 = wpool.tile([P, TPW, R, 16], f16, tag="t16")
                nc.vector.tensor_add(t16[:], t32[:, :, :, 0:16], t32[:, :, :, 16:32])
                t8 = wpool.tile([P, TPW, R, 8], f16, tag="t8")
                nc.vector.tensor_add(t8[:], t16[:, :, :, 0:8], t16[:, :, :, 8:16])
                t4 = wpool.tile([P, TPW, R, 4], f16, tag="t4")
                nc.vector.tensor_add(t4[:], t8[:, :, :, 0:4], t8[:, :, :, 4:8])
                nc.vector.tensor_reduce(
                    out=scorew[:], in_=t4[:], axis=AX.X, op=ALU.add
                )
                e_t = wpool.tile([P, TPW * R], f32, tag="et")
                nc.scalar.activation(
                    e_t[:], scorew[:].rearrange("p t r -> p (t r)"), AF.Exp
                )
                den = wpool.tile([P, TPW], f32, tag="den")
                nc.vector.tensor_reduce(
                    out=den[:],
                    in_=e_t[:].rearrange("p (t r) -> p t r", r=R),
                    axis=AX.X,
                    op=ALU.add,
                )
                nums = wpool.tile([P, TPW * R], f32, tag="nums")
                vals_bc = (
                    vals_sb[:]
                    .rearrange("p (o r) -> p o r", o=1)
                    .to_broadcast([P, TPW, R])
                )
                nc.vector.tensor_mul(
                    nums[:].rearrange("p (t r) -> p t r", r=R),
                    e_t[:].rearrange("p (t r) -> p t r", r=R),
                    vals_bc,
                )
                num = wpool.tile([P, TPW], f32, tag="num")
                nc.vector.tensor_reduce(
                    out=num[:],
                    in_=nums[:].rearrange("p (t r) -> p t r", r=R),
                    axis=AX.X,
                    op=ALU.add,
                )
                rden = wpool.tile([P, TPW], f32, tag="rden")
                nc.vector.reciprocal(rden[:], den[:])
                nc.vector.tensor_mul(
                    outbuf[:, w * TPW : (w + 1) * TPW], num[:], rden[:]
                )
            nc.sync.dma_start(out_d[:], outbuf[:])


def _build(NW, G):
    import concourse.bacc as bacc
    import concourse.mybir as mybir
    import concourse.tile as tile

    nc = bacc.Bacc(None, target_bir_lowering=False, num_swdge_queues=4)
    f32, f16, i16 = mybir.dt.float32, mybir.dt.float16, mybir.dt.int16
    TPW = NQ * G // P
    NG = NW * NQ
    ufeat_d = nc.dram_tensor("ufeat", [N_USERS, 2 * D], f16, kind="ExternalInput")
    ifT_d = nc.dram_tensor("ifT", [D, NW * P], f16, kind="ExternalInput")
    p2_d = nc.dram_tensor("p2", [D, RD], f16, kind="ExternalInput")
    idxu_d = nc.dram_tensor("idxu", [P, NG * (G // 16)], i16, kind="ExternalInput")
    selt_d = nc.dram_tensor(
        "selt", [NW * P, NQ * G], mybir.dt.float8e4, kind="ExternalInput"
    )
    vals_d = nc.dram_tensor("vals", [P, R], f32, kind="ExternalInput")
    out_d = nc.dram_tensor("out", [P, NW * TPW], f32, kind="ExternalOutput")

    with tile.TileContext(nc) as tc:
        _emit(
            nc,
            tc,
            (ufeat_d, ifT_d, p2_d, idxu_d, selt_d, vals_d, out_d),
            NW,
            G,
        )
    nc.compile()
    return nc


def _install_profile_hook():
    """Make antenv.axon_hooks available so run_bass_kernel_spmd(trace=True)
    can capture NTFF profiles through the axon .so (used by test.py only)."""
    import types

    try:
        from antenv.axon_hooks import get_axon_ntff_profile_hook  # noqa: F401

        return
    except ImportError:
        pass
    import antenv
    from trn_agent_boot.trn_boot import _ntff_profile_via_ctypes

    hook = _ntff_profile_via_ctypes("/opt/axon/libaxon_pjrt.so")
    mod = types.ModuleType("antenv.axon_hooks")
    mod._hook = hook
    mod.get_axon_ntff_profile_hook = lambda: mod._hook
    mod.set_axon_ntff_profile_hook = lambda h: setattr(mod, "_hook", h)
    sys.modules["antenv.axon_hooks"] = mod
    antenv.axon_hooks = mod


def kernel(ufeat, ifeat, Ps, src, dst):
    from concourse.bass_utils import run_bass_kernel_spmd

    ufeat = np.asarray(ufeat, np.float32)
    ifeat = np.asarray(ifeat, np.float32)
    Ps = np.asarray(Ps, np.float32)
    src = np.asarray(src, np.int32)
    dst = np.asarray(dst, np.int32)

    in_maps, metas = _prepare(ufeat, ifeat, Ps, src, dst)
    NW, G = _NC_CACHE["params"]
    key = ("nc", NW, G)
    if key not in _NC_CACHE:
        _NC_CACHE[key] = _build(NW, G)
        _NC_CACHE["nc"] = _NC_CACHE[key]
    nc = _NC_CACHE[key]
    res = run_bass_kernel_spmd(nc, in_maps, core_ids=list(range(N_CORES)))
    out = np.zeros(E, np.float32)
    for c in range(N_CORES):
        o = res.results[c]["out"]  # [P, NW*TPW]
        flat = o.T.reshape(-1)  # slot-ordered
        s2e = metas[c]
        valid = s2e >= 0
        out[s2e[valid]] = flat[valid]
    return out


# revision 21
# speedup vs baseline: 1.3047x; 1.0030x over previous
"""TRN2 Bass kernel for nn_BiDecoder (GNN edge rating decoder), 8 NeuronCores.

ratings[e] = sum_r softmax_r(ufeat[src[e]] @ Ps[r] @ ifeat[dst[e]]) * (r+1)

v2 design:
  - Edges dst-sorted into 8 contiguous shards (each core owns an item band).
  - Within a core, items are covered by NW aligned 128-item windows; edges are
    grouped by (window w, src-quarter q) and sorted by dst inside each group.
    Each group gets G slots (G multiple of 256); trailing pads idx=-1.
  - Item side needs NO gather: Y[j, (r,d)] = sum_f Ps[r,d,f] ifeat[j,f] is
    precomputed per window on PE (fp16); per tile a one-hot SelT (built with
    tensor_mask_reduce run-intervals, since dst-sorted edges give each item a
    contiguous slot run) expands Y rows to edges via one 320-col matmul.
  - User side: dma_gather of ufeat rows, 4 src-quarters (int16 idx < 25000)
    issued round-robin on 4 SWDGE queues so descriptor generation runs on all
    Q7 cpu pairs in parallel.
  - scores = sum_d us*yv via fp16 DVE mul (2x) + binary-tree adds (2x),
    softmax (no max-sub; scores bounded) -> ratings.
"""
import sys

sys.path.insert(0, "/opt/trn_rl_repo")
import numpy as np

P = 128
D = 64
R = 5
RD = R * D
N_USERS, N_ITEMS, E = 100000, 50000, 1000000
N_CORES = 8
E_CORE = E // N_CORES
NQ = 5  # overlapping int16-addressable src groups
UQ_BASE = (0, 16808, 33616, 50424, 67232)
UQ_LEN = (32768, 32768, 32768, 32768, 32768)
C_TARGET = 2520

_NC_CACHE = {}


def _plan(src, dst):
    """Shard edges; greedy equal-count windows (item span <= 128, cut mid-item
    is fine since SelT is host-built); balanced src-quarters with overlapping
    int16-addressable ranges. Returns per-core layout + global (NW, G)."""
    perm = np.argsort(dst, kind="stable")
    shards = perm.reshape(N_CORES, E_CORE)
    A = np.array(UQ_BASE)
    cores = []
    maxw = 0
    maxg = 0
    for c in range(N_CORES):
        eid = shards[c]
        d = dst[eid].astype(np.int64)
        s = src[eid].astype(np.int64)
        it0s = []
        bounds = [0]
        i = 0
        while i < E_CORE:
            it0 = int(d[i])
            j = min(
                i + C_TARGET,
                E_CORE,
                int(np.searchsorted(d, it0 + P, side="left")),
            )
            it0s.append(it0)
            bounds.append(j)
            i = j
        nw = len(it0s)
        w = np.zeros(E_CORE, np.int64)
        for wi in range(nw):
            w[bounds[wi] : bounds[wi + 1]] = wi
        it0_arr = np.array(it0s, np.int64)
        q = np.zeros(E_CORE, np.int64)
        for wi in range(nw):
            lo, hi = bounds[wi], bounds[wi + 1]
            ss = s[lo:hi]
            n = hi - lo
            q_hi = np.searchsorted(A, ss, side="right") - 1
            can_lo = (q_hi > 0) & (ss <= A[np.maximum(q_hi - 1, 0)] + 32767)
            forced = np.bincount(q_hi[~can_lo], minlength=NQ)
            flexz = np.bincount(q_hi[can_lo] - 1, minlength=NQ)[: NQ - 1]
            T = -(-n // NQ)
            qq = q_hi.copy()
            inbound = 0
            for z in range(NQ - 1):
                x = int(np.clip(T - forced[z] - inbound, 0, flexz[z]))
                idxz = np.where(can_lo & (q_hi - 1 == z))[0]
                qq[idxz[:x]] = z
                inbound = flexz[z] - x
            q[lo:hi] = qq
        g = w * NQ + q
        order = np.argsort(g, kind="stable")
        eid, d, s, w, q, g = (a[order] for a in (eid, d, s, w, q, g))
        cnt = np.bincount(g, minlength=nw * NQ)
        maxg = max(maxg, int(cnt.max()))
        maxw = max(maxw, nw)
        cores.append((eid, d, s, w, q, g, it0_arr, nw, cnt))
    NW = maxw
    G = max(128, -(-maxg // 128) * 128)
    return cores, NW, G


def _prepare(ufeat, ifeat, Ps, src, dst):
    ufeat = np.asarray(ufeat, np.float32)
    ifeat = np.asarray(ifeat, np.float32)
    Ps = np.asarray(Ps, np.float32)
    src = np.asarray(src)
    dst = np.asarray(dst)
    cores, NW, G = _plan(src, dst)
    _NC_CACHE["params"] = (NW, G)
    S16 = G // 16
    NG = NW * NQ
    p2 = np.ascontiguousarray(Ps.transpose(2, 0, 1).reshape(D, RD)).astype(np.float16)
    vals = np.tile(np.arange(1.0, 6.0, dtype=np.float32), (P, 1))
    uf = np.zeros((N_USERS, 2 * D), np.float16)
    uf[:, :D] = ufeat.astype(np.float16)
    in_maps, metas = [], []
    A = np.array(UQ_BASE)
    for (eid, d, s, w, q, g, it0_arr, nw, cnt) in cores:
        nslot = NG * G
        starts = np.zeros(NG + 1, np.int64)
        starts[1 : len(cnt) + 1] = np.cumsum(cnt)
        pos = np.arange(E_CORE) - starts[g]
        slot = g * G + pos
        idxu = np.zeros(nslot, np.int16)
        idxu[slot] = (s - A[q]).astype(np.int16)
        slot2eid = np.full(nslot, -1, np.int64)
        slot2eid[slot] = eid
        dl = d - it0_arr[w]  # 0..127 within window slab
        assert dl.min() >= 0 and dl.max() < P
        # host-built one-hot SelT (fp8): [row j, slot] = 1 iff dst-local == j
        import ml_dtypes

        selt_h = np.zeros((P, nslot), ml_dtypes.float8_e4m3fn)
        selt_h[dl, slot] = 1.0
        wrapped = (
            idxu.reshape(NG, S16, 16).transpose(2, 0, 1).reshape(16, NG * S16)
        )  # [16, (group-major, col-minor)]; element i of group = [i%16, i//16]
        wrapped = np.tile(wrapped, (8, 1)).astype(np.int16)
        slabs = np.zeros((NW * P, D), np.float32)
        for wi in range(nw):
            it0 = int(it0_arr[wi])
            navail = min(P, N_ITEMS - it0)
            slabs[wi * P : wi * P + navail] = ifeat[it0 : it0 + navail]
        ifT = np.ascontiguousarray(slabs.T).astype(np.float16)
        in_maps.append(
            {
                "ufeat": uf,
                "ifT": ifT,
                "p2": p2,
                "idxu": wrapped,
                "selt": np.ascontiguousarray(
                    selt_h.reshape(P, NW, NQ * G).transpose(1, 0, 2).reshape(
                        NW * P, NQ * G
                    )
                ),
                "vals": vals,
            }
        )
        metas.append(slot2eid)
    return in_maps, metas


def _emit(nc, tc, aps, NW, G):
    import concourse.mybir as mybir
    from concourse import library_config

    f32, f16, i16 = mybir.dt.float32, mybir.dt.float16, mybir.dt.int16
    AF = mybir.ActivationFunctionType
    ALU = mybir.AluOpType
    AX = mybir.AxisListType
    TPW = NQ * G // P
    NG = NW * NQ
    S16 = G // 16
    GT = G // P  # tiles per group
    ufeat_d, ifT_d, p2_d, idxu_d, selt_d, vals_d, out_d = aps

    nc.gpsimd.load_library(library_config.mlp)
    with tc.tile_pool(name="const", bufs=1) as cpool:
        p2_sb = cpool.tile([D, RD], f16)
        nc.sync.dma_start(p2_sb[:], p2_d[:])
        ifT_sb = cpool.tile([D, NW * P], f16)
        nc.sync.dma_start(ifT_sb[:], ifT_d[:])
        idx_sb = cpool.tile([P, NG * S16], i16)
        nc.sync.dma_start(idx_sb[:], idxu_d[:])
        vals_sb = cpool.tile([P, R], f32)
        nc.sync.dma_start(vals_sb[:], vals_d[:])
        ysb = cpool.tile([P, NW, RD], f16)
        outbuf = cpool.tile([P, NW * TPW], f32)

        with (
            tc.tile_pool(name="gather", bufs=4) as gpool,
            tc.tile_pool(name="work", bufs=2) as wpool,
            tc.tile_pool(name="psum_yv", bufs=2, space="PSUM") as zpool,
        ):

            def emit_y(wy):
                y_ps = zpool.tile([P, 512], f32, tag="y")
                nc.tensor.matmul(
                    y_ps[:, 0:RD],
                    lhsT=ifT_sb[:, wy * P : (wy + 1) * P],
                    rhs=p2_sb[:],
                )
                nc.scalar.activation(ysb[:, wy, :], y_ps[:, 0:RD], AF.Copy)

            YLOOK = 8
            for wy in range(min(YLOOK, NW)):
                emit_y(wy)
            for w in range(NW):
                if w + YLOOK < NW:
                    emit_y(w + YLOOK)
                us_f = gpool.tile([P, TPW, 2 * D], f16, tag="usf")
                for q in range(NQ):
                    gi = w * NQ + q
                    nc.gpsimd.dma_gather(
                        out_ap=us_f[:, q * GT : (q + 1) * GT, :],
                        in_ap=ufeat_d[UQ_BASE[q] : UQ_BASE[q] + UQ_LEN[q], :],
                        idxs_ap=idx_sb[:, gi * S16 : (gi + 1) * S16],
                        num_idxs=G,
                        num_idxs_reg=G,
                        elem_size=2 * D,
                        queue_num=(w * NQ + q) % 4,
                    )
                selt = gpool.tile([P, NQ * G], mybir.dt.float8e4, tag="selt")
                nc.sync.dma_start(selt[:], selt_d[w * P : (w + 1) * P, :])
                scorew = wpool.tile([P, TPW, R], f32, tag="scw")
                b24 = wpool.tile([P, TPW, R, D], f16, tag="b24")
                SUBT = 3
                nsub = -(-TPW // SUBT)
                yv_h = None
                for sub in range(nsub):
                    t0 = sub * SUBT
                    nb = min(SUBT, TPW - t0)
                    yv_ps = zpool.tile([P, SUBT, 512], f32, tag="yv")
                    for i in range(nb):
                        t = t0 + i
                        nc.tensor.matmul(
                            yv_ps[:, i, 0:RD],
                            lhsT=selt[:, t * P : (t + 1) * P],
                            rhs=ysb[:, w, :],
                        )
                    half = sub % 2
                    if half == 0:
                        yv_h = wpool.tile([P, 2 * SUBT, RD], f16, tag="yvh")
                    nc.scalar.activation(
                        yv_h[:, half * SUBT : half * SUBT + nb, :],
                        yv_ps[:, 0:nb, 0:RD],
                        AF.Copy,
                    )
                    if half == 1 or sub == nsub - 1:
                        m0 = (sub // 2) * 2 * SUBT
                        mb = t0 + nb - m0
                        nc.vector.tensor_mul(
                            b24[:, m0 : m0 + mb, :, :],
                            us_f[:, m0 : m0 + mb, 0:D]
                            .rearrange("p t (o d) -> p t o d", o=1)
                            .to_broadcast([P, mb, R, D]),
                            yv_h[:, 0:mb, :].rearrange(
                                "p t (r d) -> p t r d", r=R
                            ),
                        )
                t32 = wpool.tile([P, TPW, R, 32], f16, tag="t32")
                nc.vector.tensor_add(t32[:], b24[:, :, :, 0:32], b24[:, :, :, 32:64])
                t16 = wpool.tile([P, TPW, R, 16], f16, tag="t16")
                nc.vector.tensor_add(t16[:], t32[:, :, :, 0:16], t32[:, :, :, 16:32])
                t8 = wpool.tile([P, TPW, R, 8], f16, tag="t8")
                nc.vector.tensor_add(t8[:], t16[:, :, :, 0:8], t16[:, :, :, 8:16])
                t4 = wpool.tile([P, TPW, R, 4], f16, tag="t4")
                nc.vector.tensor_add(t4[:], t8[:, :, :, 0:4], t8[:, :, :, 4:8])
                nc.vector.tensor_reduce(
                    out=scorew[:], in_=t4[:], axis=AX.X, op=ALU.add
                )
                e_t = wpool.tile([P, TPW * R], f32, tag="et")
                nc.scalar.activation(
                    e_t[:], scorew[:].rearrange("p t r -> p (t r)"), AF.Exp
                )
                den = wpool.tile([P, TPW], f32, tag="den")
                nc.vector.tensor_reduce(
                    out=den[:],
                    in_=e_t[:].rearrange("p (t r) -> p t r", r=R),
                    axis=AX.X,
                    op=ALU.add,
                )
                nums = wpool.tile([P, TPW * R], f32, tag="nums")
                vals_bc = (
                    vals_sb[:]
                    .rearrange("p (o r) -> p o r", o=1)
                    .to_broadcast([P, TPW, R])
                )
                nc.vector.tensor_mul(
                    nums[:].rearrange("p (t r) -> p t r", r=R),
                    e_t[:].rearrange("p (t r) -> p t r", r=R),
                    vals_bc,
                )
                num = wpool.tile([P, TPW], f32, tag="num")
                nc.vector.tensor_reduce(
                    out=num[:],
                    in_=nums[:].rearrange("p (t r) -> p t r", r=R),
                    axis=AX.X,
                    op=ALU.add,
                )
                rden = wpool.tile([P, TPW], f32, tag="rden")
                nc.vector.reciprocal(rden[:], den[:])
                nc.vector.tensor_mul(
                    outbuf[:, w * TPW : (w + 1) * TPW], num[:], rden[:]
                )
            nc.sync.dma_start(out_d[:], outbuf[:])


def _build(NW, G):
    import concourse.bacc as bacc
    import concourse.mybir as mybir
    import concourse.tile as tile

    nc = bacc.Bacc(None, target_bir_lowering=False, num_swdge_queues=4)
    f32, f16, i16 = mybir.dt.float32, mybir.dt.float16, mybir.dt.int16
    TPW = NQ * G // P
    NG = NW * NQ
    ufeat_d = nc.dram_tensor("ufeat", [N_USERS, 2 * D], f16, kind="ExternalInput")
    ifT_d = nc.dram_tensor("ifT", [D, NW * P], f16, kind="ExternalInput")
    p2_d = nc.dram_tensor("p2", [D, RD], f16, kind="ExternalInput")
    idxu_d = nc.dram_tensor("idxu", [P, NG * (G // 16)], i16, kind="ExternalInput")
    selt_d = nc.dram_tensor(
        "selt", [NW * P, NQ * G], mybir.dt.float8e4, kind="ExternalInput"
    )
    vals_d = nc.dram_tensor("vals", [P, R], f32, kind="ExternalInput")
    out_d = nc.dram_tensor("out", [P, NW * TPW], f32, kind="ExternalOutput")

    with tile.TileContext(nc) as tc:
        _emit(
            nc,
            tc,
            (ufeat_d, ifT_d, p2_d, idxu_d, selt_d, vals_d, out_d),
            NW,
            G,
        )
    nc.compile()
    return nc


def _install_profile_hook():
    """Make antenv.axon_hooks available so run_bass_kernel_spmd(trace=True)
    can capture NTFF profiles through the axon .so (used by test.py only)."""
    import types

    try:
        from antenv.axon_hooks import get_axon_ntff_profile_hook  # noqa: F401

        return
    except ImportError:
        pass
    import antenv
    from trn_agent_boot.trn_boot import _ntff_profile_via_ctypes

    hook = _ntff_profile_via_ctypes("/opt/axon/libaxon_pjrt.so")
    mod = types.ModuleType("antenv.axon_hooks")
    mod._hook = hook
    mod.get_axon_ntff_profile_hook = lambda: mod._hook
    mod.set_axon_ntff_profile_hook = lambda h: setattr(mod, "_hook", h)
    sys.modules["antenv.axon_hooks"] = mod
    antenv.axon_hooks = mod


def kernel(ufeat, ifeat, Ps, src, dst):
    from concourse.bass_utils import run_bass_kernel_spmd

    ufeat = np.asarray(ufeat, np.float32)
    ifeat = np.asarray(ifeat, np.float32)
    Ps = np.asarray(Ps, np.float32)
    src = np.asarray(src, np.int32)
    dst = np.asarray(dst, np.int32)

    in_maps, metas = _prepare(ufeat, ifeat, Ps, src, dst)
    NW, G = _NC_CACHE["params"]
    key = ("nc", NW, G)
    if key not in _NC_CACHE:
        _NC_CACHE[key] = _build(NW, G)
        _NC_CACHE["nc"] = _NC_CACHE[key]
    nc = _NC_CACHE[key]
    res = run_bass_kernel_spmd(nc, in_maps, core_ids=list(range(N_CORES)))
    out = np.zeros(E, np.float32)
    for c in range(N_CORES):
        o = res.results[c]["out"]  # [P, NW*TPW]
        flat = o.T.reshape(-1)  # slot-ordered
        s2e = metas[c]
        valid = s2e >= 0
        out[s2e[valid]] = flat[valid]
    return out


# revision 22
# speedup vs baseline: 1.3412x; 1.0280x over previous
"""TRN2 Bass kernel for nn_BiDecoder (GNN edge rating decoder), 8 NeuronCores.

ratings[e] = sum_r softmax_r(ufeat[src[e]] @ Ps[r] @ ifeat[dst[e]]) * (r+1)

v2 design:
  - Edges dst-sorted into 8 contiguous shards (each core owns an item band).
  - Within a core, items are covered by NW aligned 128-item windows; edges are
    grouped by (window w, src-quarter q) and sorted by dst inside each group.
    Each group gets G slots (G multiple of 256); trailing pads idx=-1.
  - Item side needs NO gather: Y[j, (r,d)] = sum_f Ps[r,d,f] ifeat[j,f] is
    precomputed per window on PE (fp16); per tile a one-hot SelT (built with
    tensor_mask_reduce run-intervals, since dst-sorted edges give each item a
    contiguous slot run) expands Y rows to edges via one 320-col matmul.
  - User side: dma_gather of ufeat rows, 4 src-quarters (int16 idx < 25000)
    issued round-robin on 4 SWDGE queues so descriptor generation runs on all
    Q7 cpu pairs in parallel.
  - scores = sum_d us*yv via fp16 DVE mul (2x) + binary-tree adds (2x),
    softmax (no max-sub; scores bounded) -> ratings.
"""
import sys

sys.path.insert(0, "/opt/trn_rl_repo")
import numpy as np

P = 128
D = 64
R = 5
RD = R * D
N_USERS, N_ITEMS, E = 100000, 50000, 1000000
N_CORES = 8
E_CORE = E // N_CORES
NQ = 5  # overlapping int16-addressable src groups
UQ_BASE = (0, 16808, 33616, 50424, 67232)
UQ_LEN = (32768, 32768, 32768, 32768, 32768)
C_TARGET = 2520

_NC_CACHE = {}


def _plan(src, dst):
    """Shard edges; greedy equal-count windows (item span <= 128, cut mid-item
    is fine since SelT is host-built); balanced src-quarters with overlapping
    int16-addressable ranges. Returns per-core layout + global (NW, G)."""
    perm = np.argsort(dst, kind="stable")
    shards = perm.reshape(N_CORES, E_CORE)
    A = np.array(UQ_BASE)
    cores = []
    maxw = 0
    maxg = 0
    for c in range(N_CORES):
        eid = shards[c]
        d = dst[eid].astype(np.int64)
        s = src[eid].astype(np.int64)
        it0s = []
        bounds = [0]
        i = 0
        while i < E_CORE:
            it0 = int(d[i])
            j = min(
                i + C_TARGET,
                E_CORE,
                int(np.searchsorted(d, it0 + P, side="left")),
            )
            it0s.append(it0)
            bounds.append(j)
            i = j
        nw = len(it0s)
        w = np.zeros(E_CORE, np.int64)
        for wi in range(nw):
            w[bounds[wi] : bounds[wi + 1]] = wi
        it0_arr = np.array(it0s, np.int64)
        q = np.zeros(E_CORE, np.int64)
        for wi in range(nw):
            lo, hi = bounds[wi], bounds[wi + 1]
            ss = s[lo:hi]
            n = hi - lo
            q_hi = np.searchsorted(A, ss, side="right") - 1
            can_lo = (q_hi > 0) & (ss <= A[np.maximum(q_hi - 1, 0)] + 32767)
            forced = np.bincount(q_hi[~can_lo], minlength=NQ)
            flexz = np.bincount(q_hi[can_lo] - 1, minlength=NQ)[: NQ - 1]
            T = -(-n // NQ)
            qq = q_hi.copy()
            inbound = 0
            for z in range(NQ - 1):
                x = int(np.clip(T - forced[z] - inbound, 0, flexz[z]))
                idxz = np.where(can_lo & (q_hi - 1 == z))[0]
                qq[idxz[:x]] = z
                inbound = flexz[z] - x
            q[lo:hi] = qq
        g = w * NQ + q
        order = np.argsort(g, kind="stable")
        eid, d, s, w, q, g = (a[order] for a in (eid, d, s, w, q, g))
        cnt = np.bincount(g, minlength=nw * NQ)
        maxg = max(maxg, int(cnt.max()))
        maxw = max(maxw, nw)
        cores.append((eid, d, s, w, q, g, it0_arr, nw, cnt))
    NW = maxw
    G = max(128, -(-maxg // 128) * 128)
    return cores, NW, G


def _prepare(ufeat, ifeat, Ps, src, dst):
    ufeat = np.asarray(ufeat, np.float32)
    ifeat = np.asarray(ifeat, np.float32)
    Ps = np.asarray(Ps, np.float32)
    src = np.asarray(src)
    dst = np.asarray(dst)
    cores, NW, G = _plan(src, dst)
    _NC_CACHE["params"] = (NW, G)
    S16 = G // 16
    NG = NW * NQ
    p2 = np.ascontiguousarray(Ps.transpose(2, 0, 1).reshape(D, RD)).astype(np.float16)
    vals = np.tile(np.arange(1.0, 6.0, dtype=np.float32), (P, 1))
    uf = np.zeros((N_USERS, 2 * D), np.float16)
    uf[:, :D] = ufeat.astype(np.float16)
    in_maps, metas = [], []
    A = np.array(UQ_BASE)
    for (eid, d, s, w, q, g, it0_arr, nw, cnt) in cores:
        nslot = NG * G
        starts = np.zeros(NG + 1, np.int64)
        starts[1 : len(cnt) + 1] = np.cumsum(cnt)
        pos = np.arange(E_CORE) - starts[g]
        slot = g * G + pos
        idxu = np.zeros(nslot, np.int16)
        idxu[slot] = (s - A[q]).astype(np.int16)
        slot2eid = np.full(nslot, -1, np.int64)
        slot2eid[slot] = eid
        dl = d - it0_arr[w]  # 0..127 within window slab
        assert dl.min() >= 0 and dl.max() < P
        # host-built one-hot SelT (fp8): [row j, slot] = 1 iff dst-local == j
        import ml_dtypes

        selt_h = np.zeros((P, nslot), ml_dtypes.float8_e4m3fn)
        selt_h[dl, slot] = 1.0
        wrapped = (
            idxu.reshape(NG, S16, 16).transpose(2, 0, 1).reshape(16, NG * S16)
        )  # [16, (group-major, col-minor)]; element i of group = [i%16, i//16]
        wrapped = np.tile(wrapped, (8, 1)).astype(np.int16)
        slabs = np.zeros((NW * P, D), np.float32)
        for wi in range(nw):
            it0 = int(it0_arr[wi])
            navail = min(P, N_ITEMS - it0)
            slabs[wi * P : wi * P + navail] = ifeat[it0 : it0 + navail]
        ifT = np.ascontiguousarray(slabs.T).astype(np.float16)
        in_maps.append(
            {
                "ufeat": uf,
                "ifT": ifT,
                "p2": p2,
                "idxu": wrapped,
                "selt": np.ascontiguousarray(
                    selt_h.reshape(P, NW, NQ * G).transpose(1, 0, 2).reshape(
                        NW * P, NQ * G
                    )
                ),
                "vals": vals,
            }
        )
        metas.append(slot2eid)
    return in_maps, metas


def _emit(nc, tc, aps, NW, G):
    import concourse.mybir as mybir
    from concourse import library_config

    f32, f16, i16 = mybir.dt.float32, mybir.dt.float16, mybir.dt.int16
    AF = mybir.ActivationFunctionType
    ALU = mybir.AluOpType
    AX = mybir.AxisListType
    TPW = NQ * G // P
    NG = NW * NQ
    S16 = G // 16
    GT = G // P  # tiles per group
    ufeat_d, ifT_d, p2_d, idxu_d, selt_d, vals_d, out_d = aps

    nc.gpsimd.load_library(library_config.mlp)
    with tc.tile_pool(name="const", bufs=1) as cpool:
        p2_sb = cpool.tile([D, RD], f16)
        nc.sync.dma_start(p2_sb[:], p2_d[:])
        ifT_sb = cpool.tile([D, NW * P], f16)
        nc.sync.dma_start(ifT_sb[:], ifT_d[:])
        idx_sb = cpool.tile([P, NG * S16], i16)
        nc.sync.dma_start(idx_sb[:], idxu_d[:])
        vals_sb = cpool.tile([P, R], f32)
        nc.sync.dma_start(vals_sb[:], vals_d[:])
        ysb = cpool.tile([P, NW, RD], f16)
        outbuf = cpool.tile([P, NW * TPW], f32)

        with (
            tc.tile_pool(name="gather", bufs=4) as gpool,
            tc.tile_pool(name="work", bufs=2) as wpool,
            tc.tile_pool(name="psum_yv", bufs=2, space="PSUM") as zpool,
        ):

            def emit_y(wy):
                y_ps = zpool.tile([P, 4, 512], f32, tag="yv")
                nc.tensor.matmul(
                    y_ps[:, 0, 0:RD],
                    lhsT=ifT_sb[:, wy * P : (wy + 1) * P],
                    rhs=p2_sb[:],
                )
                nc.scalar.activation(ysb[:, wy, :], y_ps[:, 0, 0:RD], AF.Copy)

            YLOOK = 8
            for wy in range(min(YLOOK, NW)):
                emit_y(wy)
            for w in range(NW):
                if w + YLOOK < NW:
                    emit_y(w + YLOOK)
                us_f = gpool.tile([P, TPW, 2 * D], f16, tag="usf")
                for q in range(NQ):
                    gi = w * NQ + q
                    nc.gpsimd.dma_gather(
                        out_ap=us_f[:, q * GT : (q + 1) * GT, :],
                        in_ap=ufeat_d[UQ_BASE[q] : UQ_BASE[q] + UQ_LEN[q], :],
                        idxs_ap=idx_sb[:, gi * S16 : (gi + 1) * S16],
                        num_idxs=G,
                        num_idxs_reg=G,
                        elem_size=2 * D,
                        queue_num=(w * NQ + q) % 4,
                    )
                selt = gpool.tile([P, NQ * G], mybir.dt.float8e4, tag="selt")
                nc.sync.dma_start(selt[:], selt_d[w * P : (w + 1) * P, :])
                scorew = wpool.tile([P, TPW, R], f32, tag="scw")
                b24 = wpool.tile([P, TPW, R, D], f16, tag="b24")
                SUBT = 4
                nsub = -(-TPW // SUBT)
                yv_h = None
                for sub in range(nsub):
                    t0 = sub * SUBT
                    nb = min(SUBT, TPW - t0)
                    yv_ps = zpool.tile([P, 4, 512], f32, tag="yv")
                    for i in range(nb):
                        t = t0 + i
                        nc.tensor.matmul(
                            yv_ps[:, i, 0:RD],
                            lhsT=selt[:, t * P : (t + 1) * P],
                            rhs=ysb[:, w, :],
                        )
                    half = sub % 2
                    if half == 0:
                        yv_h = wpool.tile([P, 2 * SUBT, RD], f16, tag="yvh")
                    nc.scalar.activation(
                        yv_h[:, half * SUBT : half * SUBT + nb, :],
                        yv_ps[:, 0:nb, 0:RD],
                        AF.Copy,
                    )
                    if half == 1 or sub == nsub - 1:
                        m0 = (sub // 2) * 2 * SUBT
                        mb = t0 + nb - m0
                        nc.vector.tensor_mul(
                            b24[:, m0 : m0 + mb, :, :],
                            us_f[:, m0 : m0 + mb, 0:D]
                            .rearrange("p t (o d) -> p t o d", o=1)
                            .to_broadcast([P, mb, R, D]),
                            yv_h[:, 0:mb, :].rearrange(
                                "p t (r d) -> p t r d", r=R
                            ),
                        )
                t32 = wpool.tile([P, TPW, R, 32], f16, tag="t32")
                nc.vector.tensor_add(t32[:], b24[:, :, :, 0:32], b24[:, :, :, 32:64])
                t16 = wpool.tile([P, TPW, R, 16], f16, tag="t16")
                nc.vector.tensor_add(t16[:], t32[:, :, :, 0:16], t32[:, :, :, 16:32])
                t8 = wpool.tile([P, TPW, R, 8], f16, tag="t8")
                nc.vector.tensor_add(t8[:], t16[:, :, :, 0:8], t16[:, :, :, 8:16])
                t4 = wpool.tile([P, TPW, R, 4], f16, tag="t4")
                nc.vector.tensor_add(t4[:], t8[:, :, :, 0:4], t8[:, :, :, 4:8])
                nc.vector.tensor_reduce(
                    out=scorew[:], in_=t4[:], axis=AX.X, op=ALU.add
                )
                e_t = wpool.tile([P, TPW * R], f32, tag="et")
                nc.scalar.activation(
                    e_t[:], scorew[:].rearrange("p t r -> p (t r)"), AF.Exp
                )
                den = wpool.tile([P, TPW], f32, tag="den")
                nc.vector.tensor_reduce(
                    out=den[:],
                    in_=e_t[:].rearrange("p (t r) -> p t r", r=R),
                    axis=AX.X,
                    op=ALU.add,
                )
                nums = wpool.tile([P, TPW * R], f32, tag="nums")
                vals_bc = (
                    vals_sb[:]
                    .rearrange("p (o r) -> p o r", o=1)
                    .to_broadcast([P, TPW, R])
                )
                nc.vector.tensor_mul(
                    nums[:].rearrange("p (t r) -> p t r", r=R),
                    e_t[:].rearrange("p (t r) -> p t r", r=R),
                    vals_bc,
                )
                num = wpool.tile([P, TPW], f32, tag="num")
                nc.vector.tensor_reduce(
                    out=num[:],
                    in_=nums[:].rearrange("p (t r) -> p t r", r=R),
                    axis=AX.X,
                    op=ALU.add,
                )
                rden = wpool.tile([P, TPW], f32, tag="rden")
                nc.vector.reciprocal(rden[:], den[:])
                nc.vector.tensor_mul(
                    outbuf[:, w * TPW : (w + 1) * TPW], num[:], rden[:]
                )
            nc.sync.dma_start(out_d[:], outbuf[:])


def _build(NW, G):
    import concourse.bacc as bacc
    import concourse.mybir as mybir
    import concourse.tile as tile

    nc = bacc.Bacc(None, target_bir_lowering=False, num_swdge_queues=4)
    f32, f16, i16 = mybir.dt.float32, mybir.dt.float16, mybir.dt.int16
    TPW = NQ * G // P
    NG = NW * NQ
    ufeat_d = nc.dram_tensor("ufeat", [N_USERS, 2 * D], f16, kind="ExternalInput")
    ifT_d = nc.dram_tensor("ifT", [D, NW * P], f16, kind="ExternalInput")
    p2_d = nc.dram_tensor("p2", [D, RD], f16, kind="ExternalInput")
    idxu_d = nc.dram_tensor("idxu", [P, NG * (G // 16)], i16, kind="ExternalInput")
    selt_d = nc.dram_tensor(
        "selt", [NW * P, NQ * G], mybir.dt.float8e4, kind="ExternalInput"
    )
    vals_d = nc.dram_tensor("vals", [P, R], f32, kind="ExternalInput")
    out_d = nc.dram_tensor("out", [P, NW * TPW], f32, kind="ExternalOutput")

    with tile.TileContext(nc) as tc:
        _emit(
            nc,
            tc,
            (ufeat_d, ifT_d, p2_d, idxu_d, selt_d, vals_d, out_d),
            NW,
            G,
        )
    nc.compile()
    return nc


def _install_profile_hook():
    """Make antenv.axon_hooks available so run_bass_kernel_spmd(trace=True)
    can capture NTFF profiles through the axon .so (used by test.py only)."""
    import types

    try:
        from antenv.axon_hooks import get_axon_ntff_profile_hook  # noqa: F401

        return
    except ImportError:
        pass
    import antenv
    from trn_agent_boot.trn_boot import _ntff_profile_via_ctypes

    hook = _ntff_profile_via_ctypes("/opt/axon/libaxon_pjrt.so")
    mod = types.ModuleType("antenv.axon_hooks")
    mod._hook = hook
    mod.get_axon_ntff_profile_hook = lambda: mod._hook
    mod.set_axon_ntff_profile_hook = lambda h: setattr(mod, "_hook", h)
    sys.modules["antenv.axon_hooks"] = mod
    antenv.axon_hooks = mod


def kernel(ufeat, ifeat, Ps, src, dst):
    from concourse.bass_utils import run_bass_kernel_spmd

    ufeat = np.asarray(ufeat, np.float32)
    ifeat = np.asarray(ifeat, np.float32)
    Ps = np.asarray(Ps, np.float32)
    src = np.asarray(src, np.int32)
    dst = np.asarray(dst, np.int32)

    in_maps, metas = _prepare(ufeat, ifeat, Ps, src, dst)
    NW, G = _NC_CACHE["params"]
    key = ("nc", NW, G)
    if key not in _NC_CACHE:
        _NC_CACHE[key] = _build(NW, G)
        _NC_CACHE["nc"] = _NC_CACHE[key]
    nc = _NC_CACHE[key]
    res = run_bass_kernel_spmd(nc, in_maps, core_ids=list(range(N_CORES)))
    out = np.zeros(E, np.float32)
    for c in range(N_CORES):
        o = res.results[c]["out"]  # [P, NW*TPW]
        flat = o.T.reshape(-1)  # slot-ordered
        s2e = metas[c]
        valid = s2e >= 0
        out[s2e[valid]] = flat[valid]
    return out
